# revision 3
# baseline (speedup 1.0000x reference)
"""Trainium2 Bass kernel for MiniVandermondeKernel.

Computes kernel[h, l] = sum_p Wc[h, p] * Ac[p]^l  for l in [0, 16384),
with Ac/Wc complex (stored as (...,2) real pairs).

Strategy
--------
Shard L across the 8 cores (no collective needed): core c owns columns
[c*2048, (c+1)*2048). Within a core, split its L-range into 4 blocks of
Lb=512. Using A^(l0+d) = A^l0 * A^d, block g (global) becomes

    K[:, g*Lb + d] = (Wc * Ac^(g*Lb)) @ V0[:, d],   V0[p, d] = Ac[p]^d

so every block is a (64 x 2048) @ (2048 x 512) complex matmul against the
SAME base Vandermonde V0, with per-block modified weights. The host
precomputes V0 and the per-block weights in fp64 (cheap: ~5M elements);
the device does the 17 GFLOP contraction.

Complex matmul via PSUM accumulation with M-packing (H=64 -> M=128):
  pass 1: lhsT = [Wr^T | Wi^T]   rhs = Vr   -> psum  = [Wr@Vr ; Wi@Vr]
  pass 2: lhsT = [-Wi^T | Wr^T]  rhs = Vi   -> psum += [-Wi@Vi ; Wr@Vi]
  => psum = [Kr ; Ki]  (one PSUM bank per block, no vector epilogue)
Output DMAs straight from PSUM to DRAM.
"""
import os
import numpy as np

import concourse.bacc as bacc
import concourse.mybir as mybir
from concourse.tile import TileContext
from concourse.bass_utils import run_bass_kernel_spmd

P = 2048          # d_state
H = 64            # d_input
L = 16384         # kernel_size
NCORES = 8
LCORE = L // NCORES          # 2048 columns per core
LB = 512                     # block size (= one PSUM bank of fp32)
NBLK = LCORE // LB           # 4 blocks per core
KT = P // 128                # 16 contraction K-tiles

_DT = {
    "f32": mybir.dt.float32,
    "f32r": mybir.dt.float32r,
    "bf16": mybir.dt.bfloat16,
}


def _np_dt(dt_name):
    import ml_dtypes
    return np.dtype(ml_dtypes.bfloat16) if dt_name == "bf16" else np.float32


def build_nc(dt_name="f32", loop_iters=1):
    """Build + compile the per-core Bass program (identical on all cores)."""
    dt = _DT[dt_name]
    nc = bacc.Bacc("TRN2", target_bir_lowering=False, debug=False,
                   num_devices=NCORES)
    wl1 = nc.dram_tensor("wl1", [128, NBLK * KT * 128], dt,
                         kind="ExternalInput").ap()
    wl2 = nc.dram_tensor("wl2", [128, NBLK * KT * 128], dt,
                         kind="ExternalInput").ap()
    vr = nc.dram_tensor("vr", [128, KT * LB], dt, kind="ExternalInput").ap()
    vi = nc.dram_tensor("vi", [128, KT * LB], dt, kind="ExternalInput").ap()
    out = nc.dram_tensor("out", [128, LCORE], mybir.dt.float32,
                         kind="ExternalOutput").ap()

    with TileContext(nc) as tc:
        def body():
            with (
                tc.tile_pool(name="w", bufs=1) as wpool,
                tc.tile_pool(name="v", bufs=1) as vpool,
                tc.tile_pool(name="ps", bufs=4, space="PSUM") as pspool,
                tc.tile_pool(name="o", bufs=1) as opool,
            ):
                out_t = opool.tile([128, LCORE], mybir.dt.float32)
                wl1_t = wpool.tile([128, NBLK * KT * 128], dt)
                nc.sync.dma_start(out=wl1_t[:], in_=wl1[:])
                wl2_t = wpool.tile([128, NBLK * KT * 128], dt)
                nc.sync.dma_start(out=wl2_t[:], in_=wl2[:])
                vr_t = vpool.tile([128, KT * LB], dt)
                nc.sync.dma_start(out=vr_t[:], in_=vr[:])
                vi_t = vpool.tile([128, KT * LB], dt)
                nc.sync.dma_start(out=vi_t[:], in_=vi[:])

                for j in range(NBLK):
                    ps = pspool.tile([128, LB], mybir.dt.float32)
                    for k in range(KT):
                        base = (j * KT + k) * 128
                        nc.tensor.matmul(
                            ps[:], wl1_t[:, base:base + 128],
                            vr_t[:, k * LB:(k + 1) * LB],
                            start=(k == 0), stop=False)
                    for k in range(KT):
                        base = (j * KT + k) * 128
                        nc.tensor.matmul(
                            ps[:], wl2_t[:, base:base + 128],
                            vi_t[:, k * LB:(k + 1) * LB],
                            start=False, stop=(k == KT - 1))
                    nc.scalar.copy(out_t[:, j * LB:(j + 1) * LB], ps[:])
                nc.sync.dma_start(out=out[:], in_=out_t[:])

        if loop_iters > 1:
            with tc.For_i(0, loop_iters, 1):
                body()
        else:
            body()

    nc.compile()
    return nc


def host_prep(A, W, dt_name="f32"):
    """fp64 host-side factorization -> device input arrays.

    Returns (in_maps) for the 8 cores.
    """
    A = np.asarray(A)
    W = np.asarray(W)
    Ac = A[:, 0].astype(np.float64) + 1j * A[:, 1].astype(np.float64)
    Wc = W[..., 0].astype(np.float64) + 1j * W[..., 1].astype(np.float64)
    logA = np.log(Ac)                       # (P,) complex128
    d = np.arange(LB, dtype=np.float64)
    V0 = np.exp(logA[:, None] * d[None, :])  # (P, LB) complex128

    npdt = _np_dt(dt_name)

    def tile_rhs(x):
        # (P, LB) -> (128, KT*LB): [p_in, k*LB + d] = x[k*128 + p_in, d]
        return np.ascontiguousarray(
            x.reshape(KT, 128, LB).transpose(1, 0, 2).reshape(128, KT * LB)
        ).astype(npdt)

    Vr = tile_rhs(V0.real)
    Vi = tile_rhs(V0.imag)

    in_maps = []
    with np.errstate(under="ignore"):
        for c in range(NCORES):
            wl1 = np.empty((128, NBLK * KT * 128), npdt)
            wl2 = np.empty((128, NBLK * KT * 128), npdt)
            for j in range(NBLK):
                g = c * NBLK + j
                Bg = np.exp(logA * float(g * LB))        # (P,)
                WjT = (Wc * Bg[None, :]).T               # (P, H) complex128
                Wr = WjT.real.reshape(KT, 128, H)
                Wi = WjT.imag.reshape(KT, 128, H)
                pack1 = np.concatenate([Wr, Wi], axis=2)     # (KT,128,128)
                pack2 = np.concatenate([-Wi, Wr], axis=2)
                s = slice(j * KT * 128, (j + 1) * KT * 128)
                wl1[:, s] = pack1.transpose(1, 0, 2).reshape(128, KT * 128)
                wl2[:, s] = pack2.transpose(1, 0, 2).reshape(128, KT * 128)
            in_maps.append({"wl1": wl1, "wl2": wl2, "vr": Vr, "vi": Vi})
    return in_maps


def assemble(results):
    """Per-core (128, 2048) fp32 outputs -> (64, 16384) complex64."""
    K = np.empty((H, L), np.complex64)
    for c in range(NCORES):
        o = results[c]["out"]
        K[:, c * LCORE:(c + 1) * LCORE] = o[0:64] + 1j * o[64:128]
    return K


_compiled = {}


def _get_nc(dt_name):
    if dt_name not in _compiled:
        _compiled[dt_name] = build_nc(dt_name)
    return _compiled[dt_name]


def kernel(A, W, kernel_size):
    ks = int(np.asarray(kernel_size))
    assert ks == L, f"kernel_size {ks} != {L} (kernel is shape-specialized)"
    dt_name = os.environ.get("VDM_DT", "f32")
    nc = _get_nc(dt_name)
    in_maps = host_prep(A, W, dt_name)
    res = run_bass_kernel_spmd(nc, in_maps, core_ids=list(range(NCORES)))
    return assemble(res.results)


# revision 6
# speedup vs baseline: 1.5504x; 1.5504x over previous
"""Trainium2 Bass kernel for MiniVandermondeKernel.

Computes kernel[h, l] = sum_p Wc[h, p] * Ac[p]^l  for l in [0, 16384),
with Ac/Wc complex (stored as (...,2) real pairs).

Strategy
--------
Shard L across the 8 cores (no collective needed): core c owns columns
[c*2048, (c+1)*2048). Within a core, split its L-range into 4 blocks of
Lb=512. Using A^(l0+d) = A^l0 * A^d, block g (global) becomes

    K[:, g*Lb + d] = (Wc * Ac^(g*Lb)) @ V0[:, d],   V0[p, d] = Ac[p]^d

so every block is a (64 x 2048) @ (2048 x 512) complex matmul against the
SAME base Vandermonde V0, with per-block modified weights. The host
precomputes V0 and the per-block weights in fp64 (cheap: ~5M elements);
the device does the 17 GFLOP contraction.

Complex matmul via PSUM accumulation with M-packing (H=64 -> M=128):
  pass 1: lhsT = [Wr^T | Wi^T]   rhs = Vr   -> psum  = [Wr@Vr ; Wi@Vr]
  pass 2: lhsT = [-Wi^T | Wr^T]  rhs = Vi   -> psum += [-Wi@Vi ; Wr@Vi]
  => psum = [Kr ; Ki]  (one PSUM bank per block, no vector epilogue)

Pipelining: contraction (k) tiles stream in k-major order. Each DMA chunk
covers CK k-tiles and carries vr|vi|wl1 interleaved; the pass-2 weights
(wl2 = [-Wi|Wr]) are derived on-device from wl1 by a DVE negate + copy
(saves 1/4 of input DMA). All 4 blocks' PSUM banks accumulate in parallel,
so matmuls start as soon as the first chunk lands.
"""
import os
import numpy as np

import concourse.bacc as bacc
import concourse.mybir as mybir
from concourse.tile import TileContext
from concourse.bass_utils import run_bass_kernel_spmd

P = 2048          # d_state
H = 64            # d_input
L = 16384         # kernel_size
NCORES = 8
LCORE = L // NCORES          # 2048 columns per core
LB = 512                     # block size (= one PSUM bank of fp32)
NBLK = LCORE // LB           # 4 blocks per core
KT = P // 128                # 16 contraction K-tiles
CK = 2                       # k-tiles per DMA chunk
NCH = KT // CK               # 8 chunks
# per-chunk blob columns: CK*(LB vr + LB vi + NBLK*128 wl1)
CH_V = CK * LB               # 1024 cols of vr, then 1024 of vi
CH_W = CK * NBLK * 128       # 1024 cols of wl1
CH_COLS = 2 * CH_V + CH_W    # 3072
BLOB_COLS = NCH * CH_COLS    # 24576

_DT = {
    "f32": mybir.dt.float32,
    "f32r": mybir.dt.float32r,
    "bf16": mybir.dt.bfloat16,
}


def _np_dt(dt_name):
    import ml_dtypes
    return np.dtype(ml_dtypes.bfloat16) if dt_name == "bf16" else np.float32


def build_nc(dt_name="f32r", loop_iters=1):
    """Build + compile the per-core Bass program (identical on all cores)."""
    dt = _DT[dt_name]
    nc = bacc.Bacc("TRN2", target_bir_lowering=False, debug=False,
                   num_devices=NCORES)
    blob = nc.dram_tensor("blob", [128, BLOB_COLS], dt,
                          kind="ExternalInput").ap()
    out = nc.dram_tensor("out", [128, LCORE], mybir.dt.float32,
                         kind="ExternalOutput").ap()

    with TileContext(nc) as tc:
        def body():
            with (
                tc.tile_pool(name="bl", bufs=NCH) as blpool,
                tc.tile_pool(name="w2", bufs=NCH) as w2pool,
                tc.tile_pool(name="ps", bufs=1, space="PSUM") as pspool,
                tc.tile_pool(name="o", bufs=1) as opool,
            ):
                out_t = opool.tile([128, LCORE], mybir.dt.float32)
                ps = [pspool.tile([128, LB], mybir.dt.float32, tag=f"ps{j}",
                                  name=f"ps{j}")
                      for j in range(NBLK)]
                for s in range(NCH):
                    bl = blpool.tile([128, CH_COLS], dt)
                    nc.sync.dma_start(out=bl[:],
                                      in_=blob[:, s * CH_COLS:(s + 1) * CH_COLS])
                    # wl1 region of this chunk, viewed as (g, half, 64)
                    w1 = bl[:, 2 * CH_V:CH_COLS].rearrange(
                        "p (g two m) -> p g two m", two=2, m=64)
                    w2 = w2pool.tile([128, CH_W], dt)
                    w2v = w2.rearrange("p (g two m) -> p g two m", two=2, m=64)
                    # wl2 = [-Wi | Wr] from wl1 = [Wr | Wi]
                    nc.vector.tensor_scalar_mul(
                        w2v[:, :, 0, :], w1[:, :, 1, :], -1.0)
                    nc.vector.tensor_copy(w2v[:, :, 1, :], w1[:, :, 0, :])
                    for kk in range(CK):
                        vr_k = bl[:, kk * LB:(kk + 1) * LB]
                        vi_k = bl[:, CH_V + kk * LB:CH_V + (kk + 1) * LB]
                        first = (s == 0 and kk == 0)
                        last = (s == NCH - 1 and kk == CK - 1)
                        for j in range(NBLK):
                            wc = (kk * NBLK + j) * 128
                            nc.tensor.matmul(
                                ps[j][:], bl[:, 2 * CH_V + wc:2 * CH_V + wc + 128],
                                vr_k, start=first, stop=False)
                        for j in range(NBLK):
                            wc = (kk * NBLK + j) * 128
                            nc.tensor.matmul(
                                ps[j][:], w2[:, wc:wc + 128],
                                vi_k, start=False, stop=last)
                for j in range(NBLK):
                    nc.scalar.copy(out_t[:, j * LB:(j + 1) * LB], ps[j][:])
                nc.sync.dma_start(out=out[:], in_=out_t[:])

        if loop_iters > 1:
            with tc.For_i(0, loop_iters, 1):
                body()
        else:
            body()

    nc.compile()
    return nc


def host_prep(A, W, dt_name="f32r"):
    """fp64 host-side factorization -> per-core device input blobs."""
    A = np.asarray(A)
    W = np.asarray(W)
    Ac = A[:, 0].astype(np.float64) + 1j * A[:, 1].astype(np.float64)
    Wc = W[..., 0].astype(np.float64) + 1j * W[..., 1].astype(np.float64)
    logA = np.log(Ac)                       # (P,) complex128
    d = np.arange(LB, dtype=np.float64)
    V0 = np.exp(logA[:, None] * d[None, :])  # (P, LB) complex128

    npdt = _np_dt(dt_name)

    # (P, LB) -> (KT, 128, LB) k-tiles
    V0r = V0.real.reshape(KT, 128, LB)
    V0i = V0.imag.reshape(KT, 128, LB)

    in_maps = []
    with np.errstate(under="ignore"):
        for c in range(NCORES):
            # per-block weight packs, k-tiled: pk1[j][k] = (128, 128)
            pk1 = np.empty((NBLK, KT, 128, 128), np.float64)
            for j in range(NBLK):
                g = c * NBLK + j
                Bg = np.exp(logA * float(g * LB))        # (P,)
                WjT = (Wc * Bg[None, :]).T               # (P, H) complex128
                pk1[j, :, :, 0:H] = WjT.real.reshape(KT, 128, H)
                pk1[j, :, :, H:128] = WjT.imag.reshape(KT, 128, H)
            blob = np.empty((128, BLOB_COLS), npdt)
            for s in range(NCH):
                base = s * CH_COLS
                for kk in range(CK):
                    k = s * CK + kk
                    blob[:, base + kk * LB:base + (kk + 1) * LB] = V0r[k]
                    blob[:, base + CH_V + kk * LB:
                         base + CH_V + (kk + 1) * LB] = V0i[k]
                    for j in range(NBLK):
                        wc = base + 2 * CH_V + (kk * NBLK + j) * 128
                        blob[:, wc:wc + 128] = pk1[j, k]
            in_maps.append({"blob": blob})
    return in_maps


def assemble(results):
    """Per-core (128, 2048) fp32 outputs -> (64, 16384) complex64."""
    K = np.empty((H, L), np.complex64)
    for c in range(NCORES):
        o = results[c]["out"]
        K[:, c * LCORE:(c + 1) * LCORE] = o[0:64] + 1j * o[64:128]
    return K


_compiled = {}


def _get_nc(dt_name):
    if dt_name not in _compiled:
        _compiled[dt_name] = build_nc(dt_name)
    return _compiled[dt_name]


def kernel(A, W, kernel_size):
    ks = int(np.asarray(kernel_size))
    assert ks == L, f"kernel_size {ks} != {L} (kernel is shape-specialized)"
    dt_name = os.environ.get("VDM_DT", "f32r")
    nc = _get_nc(dt_name)
    in_maps = host_prep(A, W, dt_name)
    res = run_bass_kernel_spmd(nc, in_maps, core_ids=list(range(NCORES)))
    return assemble(res.results)


# revision 7
# speedup vs baseline: 2.3649x; 1.5254x over previous
"""Trainium2 Bass kernel for MiniVandermondeKernel.

Computes kernel[h, l] = sum_p Wc[h, p] * Ac[p]^l  for l in [0, 16384),
with Ac/Wc complex (stored as (...,2) real pairs), |Ac| in [0.9, 0.999).

Strategy
--------
INTERLEAVED L-sharding: core c owns columns l = 8t + c, t in [0, 2048).
Then kernel_c[h, t] = sum_p (Wc*Ac^c)[h,p] * B[p]^t with B = A^8 — a
Vandermonde in B, identical shape on every core (SPMD, no collective).

Within a core, split t into 4 blocks of Lb=512. B^(512j + dt) =
B^(512j) * B^dt, so block j is (Wc * A^(c + 4096j)) @ V0[:, dt] with
V0[p, dt] = B[p]^dt — every block contracts against the SAME stored V0,
with per-block host-precomputed (fp64) weights.

DECAY PRUNING: modes are sorted by |A| descending. A mode with radius r
contributes ~r^(8t); once r^(8t) < e^-C (C=32) relative to the dominant
column scale it is dropped:
  - per K-tile k (128 sorted modes), V0 columns are stored only up to
    t_k = C / (8 |ln r_max(k)|)   (rounded up to 128, capped at 512)
  - block j>0 includes K-tile k only if t_k > 512j, with the matmul N
    clipped to t_k - 512j.
This cuts input DMA ~3x and matmul work ~3x vs the dense version.

Complex matmul via PSUM accumulation with M-packing (H=64 -> M=128):
  pass 1: lhsT = [Wr^T | Wi^T]   rhs = Vr   -> psum  = [Wr@Vr ; Wi@Vr]
  pass 2: lhsT = [-Wi^T | Wr^T]  rhs = Vi   -> psum += [-Wi@Vi ; Wr@Vi]
  => psum = [Kr ; Ki]  (one PSUM bank per block, no vector epilogue)
The pass-2 weights are derived on-device from pass-1 weights by a DVE
negate + copy (saves shipping them). fp32 data is fed to the PE as
float32r (full-rate fp32 matmul).
"""
import os
import numpy as np

import concourse.bacc as bacc
import concourse.mybir as mybir
from concourse.tile import TileContext
from concourse.bass_utils import run_bass_kernel_spmd

P = 2048          # d_state
H = 64            # d_input
L = 16384         # kernel_size
NCORES = 8
TCORE = L // NCORES          # 2048 t-columns per core
LB = 512                     # block size (= one PSUM bank of fp32)
NBLK = TCORE // LB           # 4 blocks per core
KT = P // 128                # 16 contraction K-tiles
CUT = 32.0                   # drop modes past r^(8t) < e^-CUT
CHUNK_COLS = 2560            # ~1.25 MB fp32 DMA chunks

_DT = {
    "f32": mybir.dt.float32,
    "f32r": mybir.dt.float32r,
    "bf16": mybir.dt.bfloat16,
}


def _np_dt(dt_name):
    import ml_dtypes
    return np.dtype(ml_dtypes.bfloat16) if dt_name == "bf16" else np.float32


def _ceil128(x):
    return int(min(LB, 128 * np.ceil(max(x, 1) / 128)))


def make_plan(A):
    """Data-dependent pruning plan. Returns a hashable plan tuple."""
    A = np.asarray(A)
    r = np.hypot(A[:, 0].astype(np.float64), A[:, 1].astype(np.float64))
    order = np.argsort(-r)
    rs = r[order]
    # t horizon per sorted K-tile (uncapped, from its largest radius)
    t_raw = [CUT / (8.0 * max(-np.log(rs[128 * k]), 1e-12)) for k in range(KT)]
    budget = tuple(_ceil128(min(t, LB)) for t in t_raw)      # stored V0 cols
    # per block j: list of (k, n_use)
    blocks = []
    for j in range(NBLK):
        bl = []
        for k in range(KT):
            rem = t_raw[k] - LB * j
            if k == 0 or rem > 0:
                bl.append((k, _ceil128(min(rem, LB)) if k else LB))
        blocks.append(tuple(bl))
    return budget, tuple(blocks)


def _layout(plan):
    """Blob column layout: W packs first, then V tiles (vr|vi per tile)."""
    budget, blocks = plan
    wpairs = [(j, k) for j, bl in enumerate(blocks) for (k, _) in bl]
    off = {}
    col = 0
    for (j, k) in wpairs:
        off[("w", j, k)] = col
        col += 128
    w_cols = col
    for k in range(KT):
        off[("vr", k)] = col
        col += budget[k]
        off[("vi", k)] = col
        col += budget[k]
    return wpairs, off, w_cols, col


_compiled = {}


def build_nc(dt_name, plan, loop_iters=1):
    dt = _DT[dt_name]
    budget, blocks = plan
    wpairs, off, w_cols, total_cols = _layout(plan)
    nc = bacc.Bacc("TRN2", target_bir_lowering=False, debug=False,
                   num_devices=NCORES)
    blob = nc.dram_tensor("blob", [128, total_cols], dt,
                          kind="ExternalInput").ap()
    out = nc.dram_tensor("out", [128, TCORE], mybir.dt.float32,
                         kind="ExternalOutput").ap()

    # chunk boundaries: W region is chunk 0; V region split at tile bounds
    v_bounds = [w_cols]
    for k in range(KT):
        start = off[("vr", k)]
        if start + 2 * budget[k] - v_bounds[-1] >= CHUNK_COLS:
            v_bounds.append(start + 2 * budget[k])
    if v_bounds[-1] != total_cols:
        v_bounds.append(total_cols)
    vchunks = list(zip(v_bounds[:-1], v_bounds[1:]))

    def chunk_of(col):
        for i, (a, b) in enumerate(vchunks):
            if a <= col < b:
                return i
        raise ValueError(col)

    with TileContext(nc) as tc:
        def body():
            with (
                tc.tile_pool(name="wsb", bufs=1) as wpool,
                tc.tile_pool(name="vsb", bufs=1) as vpool,
                tc.tile_pool(name="ps", bufs=1, space="PSUM") as pspool,
                tc.tile_pool(name="o", bufs=1) as opool,
            ):
                out_t = opool.tile([128, TCORE], mybir.dt.float32)
                ps = [pspool.tile([128, LB], mybir.dt.float32, tag=f"ps{j}",
                                  name=f"ps{j}") for j in range(NBLK)]
                # W chunk + derived pass-2 weights
                w1t = wpool.tile([128, w_cols], dt, tag="w1", name="w1t")
                nc.sync.dma_start(out=w1t[:], in_=blob[:, 0:w_cols])
                w2t = wpool.tile([128, w_cols], dt, tag="w2", name="w2t")
                w1v = w1t.rearrange("p (g two m) -> p g two m", two=2, m=64)
                w2v = w2t.rearrange("p (g two m) -> p g two m", two=2, m=64)
                nc.vector.tensor_scalar_mul(w2v[:, :, 0, :], w1v[:, :, 1, :],
                                            -1.0)
                nc.vector.tensor_copy(w2v[:, :, 1, :], w1v[:, :, 0, :])
                # V chunks
                vt = []
                for i, (a, b) in enumerate(vchunks):
                    t = vpool.tile([128, b - a], dt, tag=f"v{i}",
                                   name=f"vt{i}")
                    eng = nc.sync if i % 2 == 0 else nc.scalar
                    eng.dma_start(out=t[:], in_=blob[:, a:b])
                    vt.append(t)

                def v_ap(kind, k, n):
                    col = off[(kind, k)]
                    i = chunk_of(col)
                    a, _ = vchunks[i]
                    return vt[i][:, col - a:col - a + n]

                started = set()
                closing = {}
                for j, bl in enumerate(blocks):
                    closing[j] = max(k for (k, _) in bl)
                for k in range(KT):
                    for j, bl in enumerate(blocks):
                        use = dict(bl).get(k)
                        if use is None:
                            continue
                        wcol = off[("w", j, k)]
                        first = j not in started
                        started.add(j)
                        last = closing[j] == k
                        nc.tensor.matmul(
                            ps[j][:, 0:use], w1t[:, wcol:wcol + 128],
                            v_ap("vr", k, use), start=first, stop=False)
                        nc.tensor.matmul(
                            ps[j][:, 0:use], w2t[:, wcol:wcol + 128],
                            v_ap("vi", k, use), start=False, stop=last)
                        if last:
                            nc.scalar.copy(out_t[:, j * LB:(j + 1) * LB],
                                           ps[j][:])
                nc.sync.dma_start(out=out[:], in_=out_t[:])

        if loop_iters > 1:
            with tc.For_i(0, loop_iters, 1):
                body()
        else:
            body()

    nc.compile()
    return nc


def host_prep(A, W, plan, dt_name):
    """fp64 host-side factorization -> per-core device input blobs."""
    budget, blocks = plan
    wpairs, off, w_cols, total_cols = _layout(plan)
    A = np.asarray(A)
    W = np.asarray(W)
    Ac = A[:, 0].astype(np.float64) + 1j * A[:, 1].astype(np.float64)
    Wc = W[..., 0].astype(np.float64) + 1j * W[..., 1].astype(np.float64)
    r = np.abs(Ac)
    order = np.argsort(-r)
    Ac = Ac[order]
    Wc = Wc[:, order]
    logA = np.log(Ac)                        # (P,) complex128
    logB = 8.0 * logA
    npdt = _np_dt(dt_name)

    # V0 tiles in B with per-tile column budgets
    vparts = {}
    for k in range(KT):
        n = budget[k]
        d = np.arange(n, dtype=np.float64)
        with np.errstate(under="ignore"):
            V = np.exp(logB[128 * k:128 * (k + 1), None] * d[None, :])
        vparts[("vr", k)] = V.real.astype(npdt)
        vparts[("vi", k)] = V.imag.astype(npdt)

    in_maps = []
    with np.errstate(under="ignore"):
        for c in range(NCORES):
            blob = np.zeros((128, total_cols), npdt)
            for (j, k) in wpairs:
                tw = np.exp(logA[128 * k:128 * (k + 1)]
                            * float(c + 8 * LB * j))
                WjT = (Wc[:, 128 * k:128 * (k + 1)] * tw[None, :]).T  # (128,H)
                col = off[("w", j, k)]
                blob[:, col:col + H] = WjT.real.astype(npdt)
                blob[:, col + H:col + 128] = WjT.imag.astype(npdt)
            for k in range(KT):
                for kind in ("vr", "vi"):
                    col = off[(kind, k)]
                    blob[:, col:col + budget[k]] = vparts[(kind, k)]
            in_maps.append({"blob": blob})
    return in_maps


def assemble(results):
    """Per-core (128, 2048) fp32 outputs -> (64, 16384) complex64."""
    K = np.empty((H, L), np.complex64)
    for c in range(NCORES):
        o = results[c]["out"]
        K[:, c::NCORES] = o[0:64] + 1j * o[64:128]
    return K


def _get_nc(dt_name, plan):
    key = (dt_name, plan)
    if key not in _compiled:
        _compiled[key] = build_nc(dt_name, plan)
    return _compiled[key]


def kernel(A, W, kernel_size):
    ks = int(np.asarray(kernel_size))
    assert ks == L, f"kernel_size {ks} != {L} (kernel is shape-specialized)"
    dt_name = os.environ.get("VDM_DT", "f32r")
    plan = make_plan(A)
    nc = _get_nc(dt_name, plan)
    in_maps = host_prep(A, W, plan, dt_name)
    res = run_bass_kernel_spmd(nc, in_maps, core_ids=list(range(NCORES)))
    return assemble(res.results)


# revision 8
# speedup vs baseline: 2.7861x; 1.1781x over previous
"""Trainium2 Bass kernel for MiniVandermondeKernel.

Computes kernel[h, l] = sum_p Wc[h, p] * Ac[p]^l  for l in [0, 16384),
with Ac/Wc complex (stored as (...,2) real pairs), |Ac| in [0.9, 0.999).

Strategy
--------
INTERLEAVED L-sharding: core c owns columns l = 8t + c, t in [0, 2048).
Then kernel_c[h, t] = sum_p (Wc*Ac^c)[h,p] * B[p]^t with B = A^8 — a
Vandermonde in B, identical shape on every core (SPMD, no collective).

Within a core, split t into 4 blocks of Lb=512. B^(512j + dt) =
B^(512j) * B^dt, so block j is (Wc * A^(c + 4096j)) @ V0[:, dt] with
V0[p, dt] = B[p]^dt — every block contracts against the SAME stored V0,
with per-block host-precomputed (fp64) weights.

DECAY PRUNING: modes are sorted by |A| descending. A mode of radius r
decays relative to the dominant column scale (~r0^(8t)) as
(r/r0)^(8t); once that ratio is < e^-C (C=18) the mode's contribution
is far below the fp32 noise floor and is dropped:
  - per K-tile k (128 sorted modes), V0 columns are stored only up to
    t_k = C / (8 (|ln r_max(k)| - |ln r0|))  (rounded up to 128, cap 512)
  - block j>0 includes K-tile k only if t_k > 512j, with the matmul N
    clipped to t_k - 512j.
This cuts input DMA ~4x and matmul work ~3x vs the dense version.

Complex matmul via PSUM accumulation with M-packing (H=64 -> M=128):
  pass 1: lhsT = [Wr^T | Wi^T]   rhs = Vr   -> psum  = [Wr@Vr ; Wi@Vr]
  pass 2: lhsT = [-Wi^T | Wr^T]  rhs = Vi   -> psum += [-Wi@Vi ; Wr@Vi]
  => psum = [Kr ; Ki]  (one PSUM bank per block, no vector epilogue)
The pass-2 weights are derived on-device from the pass-1 weights by a
DVE negate + copy (saves shipping them). fp32 data is fed to the PE as
float32r (full-rate fp32 matmul).

Blob layout / pipelining: [W(k<=1) | V0 V1 | W(k>=2) | V2.. ] in DMA
chunks of ~1.25 MB alternating over the two HWDGE rings, so matmuls
start after the first chunk lands and stream behind the DMA. Blocks
1..3 close their PSUM accumulation at small k, so their outputs DMA out
while block 0 is still contracting.
"""
import os
import numpy as np

import concourse.bacc as bacc
import concourse.mybir as mybir
from concourse.tile import TileContext
from concourse.bass_utils import run_bass_kernel_spmd

P = 2048          # d_state
H = 64            # d_input
L = 16384         # kernel_size
NCORES = 8
TCORE = L // NCORES          # 2048 t-columns per core
LB = 512                     # block size (= one PSUM bank of fp32)
NBLK = TCORE // LB           # 4 blocks per core
KT = P // 128                # 16 contraction K-tiles
CUT = 18.0                   # drop modes past (r/r0)^(8t) < e^-CUT
KSPLIT = 2                   # W packs for k < KSPLIT ship in chunk 0
CHUNK_COLS = 2560            # ~1.25 MB fp32 DMA chunk target

_DT = {
    "f32": mybir.dt.float32,
    "f32r": mybir.dt.float32r,
    "bf16": mybir.dt.bfloat16,
}


def _np_dt(dt_name):
    import ml_dtypes
    return np.dtype(ml_dtypes.bfloat16) if dt_name == "bf16" else np.float32


def _ceil128(x):
    return int(min(LB, 128 * np.ceil(max(x, 1) / 128)))


def make_plan(A):
    """Data-dependent pruning plan (hashable)."""
    A = np.asarray(A)
    r = np.hypot(A[:, 0].astype(np.float64), A[:, 1].astype(np.float64))
    rs = np.sort(r)[::-1]
    lr0 = -np.log(rs[0])
    t_raw = [CUT / (8.0 * max(-np.log(rs[128 * k]) - lr0, 1e-9))
             for k in range(KT)]
    budget = tuple(_ceil128(min(t, LB)) for t in t_raw)      # stored V0 cols
    blocks = []
    for j in range(NBLK):
        bl = []
        for k in range(KT):
            rem = t_raw[k] - LB * j
            if k == 0 or rem > 0:
                bl.append((k, _ceil128(min(rem, LB)) if k else LB))
        blocks.append(tuple(bl))
    return budget, tuple(blocks)


def _layout(plan):
    """Blob layout: sections [WA | V0..V1 | WB | V2..]; chunk boundaries.

    Returns (wpairs, off, chunks, total) where off maps entry->column,
    chunks is a list of (start, end, w_range|None), w_range = (lo, hi)
    columns of W packs inside that chunk.
    """
    budget, blocks = plan
    wpairs = sorted(
        [(j, k) for j, bl in enumerate(blocks) for (k, _) in bl],
        key=lambda jk: (jk[1], jk[0]))
    off = {}
    col = 0
    sections = []           # (cols, w_lo, w_hi or None)

    def wsec(pairs):
        nonlocal col
        lo = col
        for (j, k) in pairs:
            off[("w", j, k)] = col
            col += 128
        return (lo, col)

    def vsec(ks):
        nonlocal col
        lo = col
        for k in ks:
            off[("vr", k)] = col
            col += budget[k]
            off[("vi", k)] = col
            col += budget[k]
        return (lo, col)

    wa = wsec([jk for jk in wpairs if jk[1] < KSPLIT])
    va = vsec(range(0, KSPLIT))
    wb = wsec([jk for jk in wpairs if jk[1] >= KSPLIT])
    vb = vsec(range(KSPLIT, KT))
    total = col

    # chunks: chunk0 = [0, va.end); chunk1 = [wb.lo, wb.hi + some V);
    # then split remaining V region at ~CHUNK_COLS.
    bounds = [0, va[1]]
    cur = va[1]
    nxt = min(wb[1] + CHUNK_COLS // 2, total)
    # split the rest at v-tile boundaries
    vstarts = [off[("vr", k)] for k in range(KSPLIT, KT)] + [total]
    cuts = []
    acc = wb[1]
    for i, k in enumerate(range(KSPLIT, KT)):
        end = off[("vr", k)] + 2 * budget[k]
        if end - acc >= CHUNK_COLS or end == total:
            cuts.append(end)
            acc = end
    bounds += cuts
    if bounds[-1] != total:
        bounds.append(total)
    chunks = []
    for a, b in zip(bounds[:-1], bounds[1:]):
        w_range = None
        if a <= wa[0] < b and wa[1] > wa[0]:
            w_range = wa
        elif a <= wb[0] < b and wb[1] > wb[0]:
            w_range = wb
        chunks.append((a, b, w_range))
    return wpairs, off, chunks, total


_compiled = {}


def build_nc(dt_name, plan, loop_iters=1):
    dt = _DT[dt_name]
    budget, blocks = plan
    wpairs, off, chunks, total_cols = _layout(plan)
    nc = bacc.Bacc("TRN2", target_bir_lowering=False, debug=False,
                   num_devices=NCORES)
    blob = nc.dram_tensor("blob", [128, total_cols], dt,
                          kind="ExternalInput").ap()
    out = nc.dram_tensor("out", [128, TCORE], mybir.dt.float32,
                         kind="ExternalOutput").ap()

    def chunk_of(col):
        for i, (a, b, _) in enumerate(chunks):
            if a <= col < b:
                return i
        raise ValueError(col)

    with TileContext(nc) as tc:
        def body():
            with (
                tc.tile_pool(name="csb", bufs=1) as cpool,
                tc.tile_pool(name="wsb", bufs=1) as wpool,
                tc.tile_pool(name="ps", bufs=1, space="PSUM") as pspool,
                tc.tile_pool(name="o", bufs=1) as opool,
            ):
                out_t = opool.tile([128, TCORE], mybir.dt.float32)
                ps = [pspool.tile([128, LB], mybir.dt.float32, tag=f"ps{j}",
                                  name=f"ps{j}") for j in range(NBLK)]
                ct = []
                w2 = {}          # chunk idx -> (w2 tile, w_lo)
                for i, (a, b, w_range) in enumerate(chunks):
                    t = cpool.tile([128, b - a], dt, tag=f"c{i}",
                                   name=f"ct{i}")
                    eng = nc.sync if i % 2 == 0 else nc.scalar
                    eng.dma_start(out=t[:], in_=blob[:, a:b])
                    ct.append(t)
                    if w_range is not None:
                        lo, hi = w_range
                        g = (hi - lo) // 128
                        w2t = wpool.tile([128, hi - lo], dt, tag=f"w2{i}",
                                         name=f"w2t{i}")
                        w1v = t[:, lo - a:hi - a].rearrange(
                            "p (g two m) -> p g two m", two=2, m=64)
                        w2v = w2t.rearrange(
                            "p (g two m) -> p g two m", two=2, m=64)
                        nc.vector.tensor_scalar_mul(
                            w2v[:, :, 0, :], w1v[:, :, 1, :], -1.0)
                        nc.vector.tensor_copy(
                            w2v[:, :, 1, :], w1v[:, :, 0, :])
                        w2[i] = (w2t, lo)

                def w_aps(j, k):
                    col = off[("w", j, k)]
                    i = chunk_of(col)
                    a = chunks[i][0]
                    w2t, lo = w2[i]
                    return (ct[i][:, col - a:col - a + 128],
                            w2t[:, col - lo:col - lo + 128])

                def v_ap(kind, k, n):
                    col = off[(kind, k)]
                    i = chunk_of(col)
                    a = chunks[i][0]
                    return ct[i][:, col - a:col - a + n]

                started = set()
                closing = {j: max(k for (k, _) in bl)
                           for j, bl in enumerate(blocks)}
                for k in range(KT):
                    for j, bl in enumerate(blocks):
                        use = dict(bl).get(k)
                        if use is None:
                            continue
                        w1ap, w2ap = w_aps(j, k)
                        first = j not in started
                        started.add(j)
                        last = closing[j] == k
                        nc.tensor.matmul(
                            ps[j][:, 0:use], w1ap, v_ap("vr", k, use),
                            start=first, stop=False)
                        nc.tensor.matmul(
                            ps[j][:, 0:use], w2ap, v_ap("vi", k, use),
                            start=False, stop=last)
                        if last:
                            nc.scalar.copy(out_t[:, j * LB:(j + 1) * LB],
                                           ps[j][:])
                            eng = nc.sync if j % 2 == 0 else nc.scalar
                            eng.dma_start(
                                out=out[:, j * LB:(j + 1) * LB],
                                in_=out_t[:, j * LB:(j + 1) * LB])

        if loop_iters > 1:
            with tc.For_i(0, loop_iters, 1):
                body()
        else:
            body()

    nc.compile()
    return nc


def host_prep(A, W, plan, dt_name):
    """fp64 host-side factorization -> per-core device input blobs."""
    budget, blocks = plan
    wpairs, off, chunks, total_cols = _layout(plan)
    A = np.asarray(A)
    W = np.asarray(W)
    Ac = A[:, 0].astype(np.float64) + 1j * A[:, 1].astype(np.float64)
    Wc = W[..., 0].astype(np.float64) + 1j * W[..., 1].astype(np.float64)
    r = np.abs(Ac)
    order = np.argsort(-r)
    Ac = Ac[order]
    Wc = Wc[:, order]
    logA = np.log(Ac)                        # (P,) complex128
    logB = 8.0 * logA
    npdt = _np_dt(dt_name)

    vparts = {}
    for k in range(KT):
        n = budget[k]
        d = np.arange(n, dtype=np.float64)
        with np.errstate(under="ignore"):
            V = np.exp(logB[128 * k:128 * (k + 1), None] * d[None, :])
        vparts[("vr", k)] = V.real.astype(npdt)
        vparts[("vi", k)] = V.imag.astype(npdt)

    in_maps = []
    with np.errstate(under="ignore"):
        for c in range(NCORES):
            blob = np.zeros((128, total_cols), npdt)
            for (j, k) in wpairs:
                tw = np.exp(logA[128 * k:128 * (k + 1)]
                            * float(c + 8 * LB * j))
                WjT = (Wc[:, 128 * k:128 * (k + 1)] * tw[None, :]).T  # (128,H)
                col = off[("w", j, k)]
                blob[:, col:col + H] = WjT.real.astype(npdt)
                blob[:, col + H:col + 128] = WjT.imag.astype(npdt)
            for k in range(KT):
                for kind in ("vr", "vi"):
                    col = off[(kind, k)]
                    blob[:, col:col + budget[k]] = vparts[(kind, k)]
            in_maps.append({"blob": blob})
    return in_maps


def assemble(results):
    """Per-core (128, 2048) fp32 outputs -> (64, 16384) complex64."""
    K = np.empty((H, L), np.complex64)
    for c in range(NCORES):
        o = results[c]["out"]
        K[:, c::NCORES] = o[0:64] + 1j * o[64:128]
    return K


def _get_nc(dt_name, plan):
    key = (dt_name, plan)
    if key not in _compiled:
        _compiled[key] = build_nc(dt_name, plan)
    return _compiled[key]


def kernel(A, W, kernel_size):
    ks = int(np.asarray(kernel_size))
    assert ks == L, f"kernel_size {ks} != {L} (kernel is shape-specialized)"
    dt_name = os.environ.get("VDM_DT", "f32r")
    plan = make_plan(A)
    nc = _get_nc(dt_name, plan)
    in_maps = host_prep(A, W, plan, dt_name)
    res = run_bass_kernel_spmd(nc, in_maps, core_ids=list(range(NCORES)))
    return assemble(res.results)


# revision 9
# speedup vs baseline: 3.0717x; 1.1025x over previous
"""Trainium2 Bass kernel for MiniVandermondeKernel.

Computes kernel[h, l] = sum_p Wc[h, p] * Ac[p]^l  for l in [0, 16384),
with Ac/Wc complex (stored as (...,2) real pairs), |Ac| in [0.9, 0.999).

Strategy
--------
INTERLEAVED L-sharding: core c owns columns l = 8t + c, t in [0, 2048).
Then kernel_c[h, t] = sum_p (Wc*Ac^c)[h,p] * B[p]^t with B = A^8 — a
Vandermonde in B, identical shape on every core (SPMD, no collective).

Within a core, split t into 4 blocks of Lb=512. B^(512j + dt) =
B^(512j) * B^dt, so block j is (Wc * A^(c + 4096j)) @ V0[:, dt] with
V0[p, dt] = B[p]^dt — every block contracts against the SAME stored V0,
with per-block host-precomputed (fp64) weights.

DECAY PRUNING: modes are sorted by |A| descending. A mode of radius r
decays relative to the dominant column scale (~r0^(8t)) as
(r/r0)^(8t); once that ratio is < e^-C (C=18) the mode's contribution
is far below the fp32 noise floor and is dropped:
  - per K-tile k (128 sorted modes), V0 columns are stored only up to
    t_k = C / (8 (|ln r_max(k)| - |ln r0|))  (rounded up to 128, cap 512)
  - block j>0 includes K-tile k only if t_k > 512j, with the matmul N
    clipped to t_k - 512j.
This cuts input DMA ~4x and matmul work ~3x vs the dense version.

Complex matmul via PSUM accumulation with M-packing (H=64 -> M=128):
  pass 1: lhsT = [Wr^T | Wi^T]   rhs = Vr   -> psum  = [Wr@Vr ; Wi@Vr]
  pass 2: lhsT = [-Wi^T | Wr^T]  rhs = Vi   -> psum += [-Wi@Vi ; Wr@Vi]
  => psum = [Kr ; Ki]  (one PSUM bank per block, no vector epilogue)
The pass-2 weights are derived on-device from the pass-1 weights by a
DVE negate + copy (saves shipping them). fp32 data is fed to the PE as
float32r (full-rate fp32 matmul).

Blob layout / pipelining: [W(k<=1) | V0 V1 | W(k>=2) | V2.. ] in DMA
chunks of ~1.25 MB alternating over the two HWDGE rings, so matmuls
start after the first chunk lands and stream behind the DMA. Blocks
1..3 close their PSUM accumulation at small k, so their outputs DMA out
while block 0 is still contracting.
"""
import os
import numpy as np

import concourse.bacc as bacc
import concourse.mybir as mybir
from concourse.tile import TileContext
from concourse.bass_utils import run_bass_kernel_spmd

P = 2048          # d_state
H = 64            # d_input
L = 16384         # kernel_size
NCORES = 8
TCORE = L // NCORES          # 2048 t-columns per core
LB = 512                     # block size (= one PSUM bank of fp32)
NBLK = TCORE // LB           # 4 blocks per core
KT = P // 128                # 16 contraction K-tiles
CUT = 18.0                   # drop modes past (r/r0)^(8t) < e^-CUT
KSPLIT = 2                   # W packs for k < KSPLIT ship in chunk 0
CHUNK_COLS = 2240            # ~1.1 MB fp32 DMA chunk target

_DT = {
    "f32": mybir.dt.float32,
    "f32r": mybir.dt.float32r,
    "bf16": mybir.dt.bfloat16,
}


def _np_dt(dt_name):
    import ml_dtypes
    return np.dtype(ml_dtypes.bfloat16) if dt_name == "bf16" else np.float32


def _ceil64(x):
    return int(min(LB, 64 * np.ceil(max(x, 1) / 64)))


def make_plan(A):
    """Data-dependent pruning plan (hashable)."""
    A = np.asarray(A)
    r = np.hypot(A[:, 0].astype(np.float64), A[:, 1].astype(np.float64))
    rs = np.sort(r)[::-1]
    lr0 = -np.log(rs[0])
    t_raw = [CUT / (8.0 * max(-np.log(rs[128 * k]) - lr0, 1e-9))
             for k in range(KT)]
    budget = tuple(_ceil64(min(t, LB)) for t in t_raw)      # stored V0 cols
    blocks = []
    for j in range(NBLK):
        bl = []
        for k in range(KT):
            rem = t_raw[k] - LB * j
            if k == 0 or rem > 0:
                bl.append((k, _ceil64(min(rem, LB)) if k else LB))
        blocks.append(tuple(bl))
    return budget, tuple(blocks)


def _layout(plan):
    """Blob layout: sections [WA | V0..V1 | WB | V2..]; chunk boundaries.

    Returns (wpairs, off, chunks, total) where off maps entry->column,
    chunks is a list of (start, end, w_range|None), w_range = (lo, hi)
    columns of W packs inside that chunk.
    """
    budget, blocks = plan
    wpairs = sorted(
        [(j, k) for j, bl in enumerate(blocks) for (k, _) in bl],
        key=lambda jk: (jk[1], jk[0]))
    off = {}
    col = 0
    sections = []           # (cols, w_lo, w_hi or None)

    def wsec(pairs):
        nonlocal col
        lo = col
        for (j, k) in pairs:
            off[("w", j, k)] = col
            col += 128
        return (lo, col)

    def vsec(ks):
        nonlocal col
        lo = col
        for k in ks:
            off[("vr", k)] = col
            col += budget[k]
            off[("vi", k)] = col
            col += budget[k]
        return (lo, col)

    wa = wsec([jk for jk in wpairs if jk[1] < KSPLIT])
    va = vsec(range(0, KSPLIT))
    wb = wsec([jk for jk in wpairs if jk[1] >= KSPLIT])
    vb = vsec(range(KSPLIT, KT))
    total = col

    # chunks: chunk0 = [0, va.end); chunk1 = [wb.lo, wb.hi + some V);
    # then split remaining V region at ~CHUNK_COLS.
    bounds = [0, va[1]]
    acc = va[1]
    cuts = []
    for k in range(KSPLIT, KT):
        end = off[("vr", k)] + 2 * budget[k]
        if end - acc >= CHUNK_COLS or end == total:
            cuts.append(end)
            acc = end
    bounds += cuts
    if bounds[-1] != total:
        bounds.append(total)
    chunks = []
    for a, b in zip(bounds[:-1], bounds[1:]):
        w_range = None
        if a <= wa[0] < b and wa[1] > wa[0]:
            w_range = wa
        elif a <= wb[0] < b and wb[1] > wb[0]:
            w_range = wb
        chunks.append((a, b, w_range))
    return wpairs, off, chunks, total


_compiled = {}


def build_nc(dt_name, plan, loop_iters=1):
    dt = _DT[dt_name]
    budget, blocks = plan
    wpairs, off, chunks, total_cols = _layout(plan)
    nc = bacc.Bacc("TRN2", target_bir_lowering=False, debug=False,
                   num_devices=NCORES)
    blob = nc.dram_tensor("blob", [128, total_cols], dt,
                          kind="ExternalInput").ap()
    out = nc.dram_tensor("out", [128, TCORE], mybir.dt.float32,
                         kind="ExternalOutput").ap()

    def chunk_of(col):
        for i, (a, b, _) in enumerate(chunks):
            if a <= col < b:
                return i
        raise ValueError(col)

    with TileContext(nc) as tc:
        def body():
            with (
                tc.tile_pool(name="csb", bufs=1) as cpool,
                tc.tile_pool(name="wsb", bufs=1) as wpool,
                tc.tile_pool(name="ps", bufs=1, space="PSUM") as pspool,
                tc.tile_pool(name="o", bufs=1) as opool,
            ):
                out_t = opool.tile([128, TCORE], mybir.dt.float32)
                ps = [pspool.tile([128, LB], mybir.dt.float32, tag=f"ps{j}",
                                  name=f"ps{j}") for j in range(NBLK)]
                ct = []
                w2 = {}          # chunk idx -> (w2 tile, w_lo)
                for i, (a, b, w_range) in enumerate(chunks):
                    t = cpool.tile([128, b - a], dt, tag=f"c{i}",
                                   name=f"ct{i}")
                    eng = nc.sync if i % 2 == 0 else nc.scalar
                    eng.dma_start(out=t[:], in_=blob[:, a:b])
                    ct.append(t)
                    if w_range is not None:
                        lo, hi = w_range
                        g = (hi - lo) // 128
                        w2t = wpool.tile([128, hi - lo], dt, tag=f"w2{i}",
                                         name=f"w2t{i}")
                        w1v = t[:, lo - a:hi - a].rearrange(
                            "p (g two m) -> p g two m", two=2, m=64)
                        w2v = w2t.rearrange(
                            "p (g two m) -> p g two m", two=2, m=64)
                        nc.vector.tensor_scalar_mul(
                            w2v[:, :, 0, :], w1v[:, :, 1, :], -1.0)
                        nc.vector.tensor_copy(
                            w2v[:, :, 1, :], w1v[:, :, 0, :])
                        w2[i] = (w2t, lo)

                def w_aps(j, k):
                    col = off[("w", j, k)]
                    i = chunk_of(col)
                    a = chunks[i][0]
                    w2t, lo = w2[i]
                    return (ct[i][:, col - a:col - a + 128],
                            w2t[:, col - lo:col - lo + 128])

                def v_ap(kind, k, n):
                    col = off[(kind, k)]
                    i = chunk_of(col)
                    a = chunks[i][0]
                    return ct[i][:, col - a:col - a + n]

                started = set()
                closing = {j: max(k for (k, _) in bl)
                           for j, bl in enumerate(blocks)}
                for k in range(KT):
                    for j, bl in enumerate(blocks):
                        use = dict(bl).get(k)
                        if use is None:
                            continue
                        w1ap, w2ap = w_aps(j, k)
                        first = j not in started
                        started.add(j)
                        last = closing[j] == k
                        nc.tensor.matmul(
                            ps[j][:, 0:use], w1ap, v_ap("vr", k, use),
                            start=first, stop=False)
                        nc.tensor.matmul(
                            ps[j][:, 0:use], w2ap, v_ap("vi", k, use),
                            start=False, stop=last)
                        if last:
                            nc.scalar.copy(out_t[:, j * LB:(j + 1) * LB],
                                           ps[j][:])
                            eng = nc.sync if j % 2 == 0 else nc.scalar
                            eng.dma_start(
                                out=out[:, j * LB:(j + 1) * LB],
                                in_=out_t[:, j * LB:(j + 1) * LB])

        if loop_iters > 1:
            with tc.For_i(0, loop_iters, 1):
                body()
        else:
            body()

    nc.compile()
    return nc


def host_prep(A, W, plan, dt_name):
    """fp64 host-side factorization -> per-core device input blobs."""
    budget, blocks = plan
    wpairs, off, chunks, total_cols = _layout(plan)
    A = np.asarray(A)
    W = np.asarray(W)
    Ac = A[:, 0].astype(np.float64) + 1j * A[:, 1].astype(np.float64)
    Wc = W[..., 0].astype(np.float64) + 1j * W[..., 1].astype(np.float64)
    r = np.abs(Ac)
    order = np.argsort(-r)
    Ac = Ac[order]
    Wc = Wc[:, order]
    logA = np.log(Ac)                        # (P,) complex128
    logB = 8.0 * logA
    npdt = _np_dt(dt_name)

    vparts = {}
    for k in range(KT):
        n = budget[k]
        d = np.arange(n, dtype=np.float64)
        with np.errstate(under="ignore"):
            V = np.exp(logB[128 * k:128 * (k + 1), None] * d[None, :])
        vparts[("vr", k)] = V.real.astype(npdt)
        vparts[("vi", k)] = V.imag.astype(npdt)

    in_maps = []
    with np.errstate(under="ignore"):
        for c in range(NCORES):
            blob = np.zeros((128, total_cols), npdt)
            for (j, k) in wpairs:
                tw = np.exp(logA[128 * k:128 * (k + 1)]
                            * float(c + 8 * LB * j))
                WjT = (Wc[:, 128 * k:128 * (k + 1)] * tw[None, :]).T  # (128,H)
                col = off[("w", j, k)]
                blob[:, col:col + H] = WjT.real.astype(npdt)
                blob[:, col + H:col + 128] = WjT.imag.astype(npdt)
            for k in range(KT):
                for kind in ("vr", "vi"):
                    col = off[(kind, k)]
                    blob[:, col:col + budget[k]] = vparts[(kind, k)]
            in_maps.append({"blob": blob})
    return in_maps


def assemble(results):
    """Per-core (128, 2048) fp32 outputs -> (64, 16384) complex64."""
    K = np.empty((H, L), np.complex64)
    for c in range(NCORES):
        o = results[c]["out"]
        K[:, c::NCORES] = o[0:64] + 1j * o[64:128]
    return K


def _get_nc(dt_name, plan):
    key = (dt_name, plan)
    if key not in _compiled:
        _compiled[key] = build_nc(dt_name, plan)
    return _compiled[key]


def kernel(A, W, kernel_size):
    ks = int(np.asarray(kernel_size))
    assert ks == L, f"kernel_size {ks} != {L} (kernel is shape-specialized)"
    dt_name = os.environ.get("VDM_DT", "f32r")
    plan = make_plan(A)
    nc = _get_nc(dt_name, plan)
    in_maps = host_prep(A, W, plan, dt_name)
    res = run_bass_kernel_spmd(nc, in_maps, core_ids=list(range(NCORES)))
    return assemble(res.results)


# revision 10
# speedup vs baseline: 3.2229x; 1.0492x over previous
"""Trainium2 Bass kernel for MiniVandermondeKernel.

Computes kernel[h, l] = sum_p Wc[h, p] * Ac[p]^l  for l in [0, 16384),
with Ac/Wc complex (stored as (...,2) real pairs), |Ac| in [0.9, 0.999).

Strategy
--------
INTERLEAVED L-sharding: core c owns columns l = 8t + c, t in [0, 2048).
Then kernel_c[h, t] = sum_p (Wc*Ac^c)[h,p] * B[p]^t with B = A^8 — a
Vandermonde in B, identical shape on every core (SPMD, no collective).

Within a core, split t into 4 blocks of Lb=512. B^(512j + dt) =
B^(512j) * B^dt, so block j is (Wc * A^(c + 4096j)) @ V0[:, dt] with
V0[p, dt] = B[p]^dt — every block contracts against the SAME stored V0,
with per-block host-precomputed (fp64) weights.

DECAY PRUNING: modes are sorted by |A| descending. A mode of radius r
decays relative to the dominant column scale (~r0^(8t)) as
(r/r0)^(8t); once that ratio is < e^-C (C=18) the mode's contribution
is far below the fp32 noise floor and is dropped:
  - per K-tile k (128 sorted modes), V0 columns are stored only up to
    t_k = C / (8 (|ln r_max(k)| - |ln r0|))  (rounded up to 128, cap 512)
  - block j>0 includes K-tile k only if t_k > 512j, with the matmul N
    clipped to t_k - 512j.
This cuts input DMA ~4x and matmul work ~3x vs the dense version.

Complex matmul via PSUM accumulation with M-packing (H=64 -> M=128):
  pass 1: lhsT = [Wr^T | Wi^T]   rhs = Vr   -> psum  = [Wr@Vr ; Wi@Vr]
  pass 2: lhsT = [-Wi^T | Wr^T]  rhs = Vi   -> psum += [-Wi@Vi ; Wr@Vi]
  => psum = [Kr ; Ki]  (one PSUM bank per block, no vector epilogue)
The pass-2 weights are derived on-device from the pass-1 weights by a
DVE negate + copy (saves shipping them). fp32 data is fed to the PE as
float32r (full-rate fp32 matmul).

Blob layout / pipelining: [W(k<=1) | V0 V1 | W(k>=2) | V2.. ] in DMA
chunks of ~1.25 MB alternating over the two HWDGE rings, so matmuls
start after the first chunk lands and stream behind the DMA. Blocks
1..3 close their PSUM accumulation at small k, so their outputs DMA out
while block 0 is still contracting.
"""
import os
import numpy as np

import concourse.bacc as bacc
import concourse.mybir as mybir
from concourse.tile import TileContext
from concourse.bass_utils import run_bass_kernel_spmd

P = 2048          # d_state
H = 64            # d_input
L = 16384         # kernel_size
NCORES = 8
TCORE = L // NCORES          # 2048 t-columns per core
LB = 512                     # block size (= one PSUM bank of fp32)
NBLK = TCORE // LB           # 4 blocks per core
KT = P // 128                # 16 contraction K-tiles
CUT = 18.0                   # drop modes past (r/r0)^(8t) < e^-CUT
KSPLIT = 2                   # W packs for k < KSPLIT ship in chunk 0
CHUNK_COLS = 1100            # ~550 KB fp32 DMA chunk target

_DT = {
    "f32": mybir.dt.float32,
    "f32r": mybir.dt.float32r,
    "bf16": mybir.dt.bfloat16,
}


def _np_dt(dt_name):
    import ml_dtypes
    return np.dtype(ml_dtypes.bfloat16) if dt_name == "bf16" else np.float32


def _ceil64(x):
    return int(min(LB, 64 * np.ceil(max(x, 1) / 64)))


def make_plan(A):
    """Data-dependent pruning plan (hashable)."""
    A = np.asarray(A)
    r = np.hypot(A[:, 0].astype(np.float64), A[:, 1].astype(np.float64))
    rs = np.sort(r)[::-1]
    lr0 = -np.log(rs[0])
    t_raw = [CUT / (8.0 * max(-np.log(rs[128 * k]) - lr0, 1e-9))
             for k in range(KT)]
    budget = tuple(_ceil64(min(t, LB)) for t in t_raw)      # stored V0 cols
    blocks = []
    for j in range(NBLK):
        bl = []
        for k in range(KT):
            rem = t_raw[k] - LB * j
            if k == 0 or rem > 0:
                bl.append((k, _ceil64(min(rem, LB)) if k else LB))
        blocks.append(tuple(bl))
    return budget, tuple(blocks)


def _layout(plan):
    """Blob layout: k-major entry list  [W packs for k | vr_k | vi_k] ...

    Returns (wpairs, off, chunks, total). chunks is a list of
    (start, end, wruns) where wruns is a list of (lo, hi) column ranges
    of W packs inside the chunk.
    """
    budget, blocks = plan
    wpairs = sorted(
        [(j, k) for j, bl in enumerate(blocks) for (k, _) in bl],
        key=lambda jk: (jk[1], jk[0]))
    off = {}
    entries = []             # (start_col, end_col, is_w)
    col = 0
    for k in range(KT):
        for (j, kk) in wpairs:
            if kk == k:
                off[("w", j, k)] = col
                entries.append((col, col + 128, True))
                col += 128
        off[("vr", k)] = col
        entries.append((col, col + budget[k], False))
        col += budget[k]
        off[("vi", k)] = col
        entries.append((col, col + budget[k], False))
        col += budget[k]
    total = col

    chunks = []
    start = 0
    wruns = []
    run = None
    for (a, b, is_w) in entries:
        if is_w:
            if run is not None and run[1] == a:
                run = (run[0], b)
            else:
                if run is not None:
                    wruns.append(run)
                run = (a, b)
        else:
            if run is not None:
                wruns.append(run)
                run = None
        if b - start >= CHUNK_COLS or b == total:
            if run is not None:       # close an open W run at chunk edge
                wruns.append((run[0], b))
                run = (b, b) if b != total else None
                if run is not None and run[0] == run[1]:
                    run = None
            chunks.append((start, b, [r for r in wruns if r[1] > r[0]]))
            start = b
            wruns = []
    return wpairs, off, chunks, total


_compiled = {}


def build_nc(dt_name, plan, loop_iters=1):
    dt = _DT[dt_name]
    budget, blocks = plan
    wpairs, off, chunks, total_cols = _layout(plan)
    nc = bacc.Bacc("TRN2", target_bir_lowering=False, debug=False,
                   num_devices=NCORES)
    blob = nc.dram_tensor("blob", [128, total_cols], dt,
                          kind="ExternalInput").ap()
    out = nc.dram_tensor("out", [128, TCORE], mybir.dt.float32,
                         kind="ExternalOutput").ap()

    def chunk_of(col):
        for i, (a, b, _) in enumerate(chunks):
            if a <= col < b:
                return i
        raise ValueError(col)

    with TileContext(nc) as tc:
        def body():
            with (
                tc.tile_pool(name="csb", bufs=1) as cpool,
                tc.tile_pool(name="wsb", bufs=1) as wpool,
                tc.tile_pool(name="ps", bufs=1, space="PSUM") as pspool,
                tc.tile_pool(name="o", bufs=1) as opool,
            ):
                out_t = opool.tile([128, TCORE], mybir.dt.float32)
                ps = [pspool.tile([128, LB], mybir.dt.float32, tag=f"ps{j}",
                                  name=f"ps{j}") for j in range(NBLK)]
                ct = []
                w2 = {}          # (run_lo) -> (w2 tile, run_lo)
                for i, (a, b, wruns) in enumerate(chunks):
                    t = cpool.tile([128, b - a], dt, tag=f"c{i}",
                                   name=f"ct{i}")
                    eng = nc.sync if i % 2 == 0 else nc.scalar
                    eng.dma_start(out=t[:], in_=blob[:, a:b])
                    ct.append(t)
                    for (lo, hi) in wruns:
                        w2t = wpool.tile([128, hi - lo], dt,
                                         tag=f"w2_{lo}", name=f"w2t{lo}")
                        w1v = t[:, lo - a:hi - a].rearrange(
                            "p (g two m) -> p g two m", two=2, m=64)
                        w2v = w2t.rearrange(
                            "p (g two m) -> p g two m", two=2, m=64)
                        nc.vector.tensor_scalar_mul(
                            w2v[:, :, 0, :], w1v[:, :, 1, :], -1.0)
                        nc.vector.tensor_copy(
                            w2v[:, :, 1, :], w1v[:, :, 0, :])
                        w2[lo] = w2t

                def w_aps(j, k):
                    col = off[("w", j, k)]
                    i = chunk_of(col)
                    a = chunks[i][0]
                    for (lo, hi) in chunks[i][2]:
                        if lo <= col < hi:
                            return (ct[i][:, col - a:col - a + 128],
                                    w2[lo][:, col - lo:col - lo + 128])
                    raise ValueError((j, k))

                def v_ap(kind, k, n):
                    col = off[(kind, k)]
                    i = chunk_of(col)
                    a = chunks[i][0]
                    return ct[i][:, col - a:col - a + n]

                started = set()
                closing = {j: max(k for (k, _) in bl)
                           for j, bl in enumerate(blocks)}
                for k in range(KT):
                    for j, bl in enumerate(blocks):
                        use = dict(bl).get(k)
                        if use is None:
                            continue
                        w1ap, w2ap = w_aps(j, k)
                        first = j not in started
                        started.add(j)
                        last = closing[j] == k
                        nc.tensor.matmul(
                            ps[j][:, 0:use], w1ap, v_ap("vr", k, use),
                            start=first, stop=False)
                        nc.tensor.matmul(
                            ps[j][:, 0:use], w2ap, v_ap("vi", k, use),
                            start=False, stop=last)
                        if last:
                            nc.scalar.copy(out_t[:, j * LB:(j + 1) * LB],
                                           ps[j][:])
                            eng = nc.sync if j % 2 == 0 else nc.scalar
                            eng.dma_start(
                                out=out[:, j * LB:(j + 1) * LB],
                                in_=out_t[:, j * LB:(j + 1) * LB])

        if loop_iters > 1:
            with tc.For_i(0, loop_iters, 1):
                body()
        else:
            body()

    nc.compile()
    return nc


def host_prep(A, W, plan, dt_name):
    """fp64 host-side factorization -> per-core device input blobs."""
    budget, blocks = plan
    wpairs, off, chunks, total_cols = _layout(plan)
    A = np.asarray(A)
    W = np.asarray(W)
    Ac = A[:, 0].astype(np.float64) + 1j * A[:, 1].astype(np.float64)
    Wc = W[..., 0].astype(np.float64) + 1j * W[..., 1].astype(np.float64)
    r = np.abs(Ac)
    order = np.argsort(-r)
    Ac = Ac[order]
    Wc = Wc[:, order]
    logA = np.log(Ac)                        # (P,) complex128
    logB = 8.0 * logA
    npdt = _np_dt(dt_name)

    vparts = {}
    for k in range(KT):
        n = budget[k]
        d = np.arange(n, dtype=np.float64)
        with np.errstate(under="ignore"):
            V = np.exp(logB[128 * k:128 * (k + 1), None] * d[None, :])
        vparts[("vr", k)] = V.real.astype(npdt)
        vparts[("vi", k)] = V.imag.astype(npdt)

    in_maps = []
    with np.errstate(under="ignore"):
        for c in range(NCORES):
            blob = np.zeros((128, total_cols), npdt)
            for (j, k) in wpairs:
                tw = np.exp(logA[128 * k:128 * (k + 1)]
                            * float(c + 8 * LB * j))
                WjT = (Wc[:, 128 * k:128 * (k + 1)] * tw[None, :]).T  # (128,H)
                col = off[("w", j, k)]
                blob[:, col:col + H] = WjT.real.astype(npdt)
                blob[:, col + H:col + 128] = WjT.imag.astype(npdt)
            for k in range(KT):
                for kind in ("vr", "vi"):
                    col = off[(kind, k)]
                    blob[:, col:col + budget[k]] = vparts[(kind, k)]
            in_maps.append({"blob": blob})
    return in_maps


def assemble(results):
    """Per-core (128, 2048) fp32 outputs -> (64, 16384) complex64."""
    K = np.empty((H, L), np.complex64)
    for c in range(NCORES):
        o = results[c]["out"]
        K[:, c::NCORES] = o[0:64] + 1j * o[64:128]
    return K


def _get_nc(dt_name, plan):
    key = (dt_name, plan)
    if key not in _compiled:
        _compiled[key] = build_nc(dt_name, plan)
    return _compiled[key]


def kernel(A, W, kernel_size):
    ks = int(np.asarray(kernel_size))
    assert ks == L, f"kernel_size {ks} != {L} (kernel is shape-specialized)"
    dt_name = os.environ.get("VDM_DT", "f32r")
    plan = make_plan(A)
    nc = _get_nc(dt_name, plan)
    in_maps = host_prep(A, W, plan, dt_name)
    res = run_bass_kernel_spmd(nc, in_maps, core_ids=list(range(NCORES)))
    return assemble(res.results)
